# revision 8
# baseline (speedup 1.0000x reference)
"""Trainium2 Bass kernel for the CellularAutomata step (dense_cnn) — v2.

Math (per pixel): s = depthwise3x3(wrap_pad(x), [identity, sobel_x, sobel_y]);
h = relu(s @ W1 + b1); out = clip(x + h @ W2 + b2, 0, 1).

Strategy (pure data parallel, batch -> 8 cores, weights replicated):
  - Host: per-core image to channel-major flat layout [16, 258*258] with wrap
    padding (bf16); the device computes the padded flat grid and the host
    slices out the valid 256x256 region.
  - The 3x3 conv + W1 are folded host-side.  xx holds 6 shifted copies of x
    on 96 partitions: partition dy*48+dx*16+c = x[c, q0+dy*258+dx+f], loaded
    in one DMA.  Layer1 for a 1024-px pair is TWO bf16 matmuls: K=96 at
    column f0 (taps dy=0,1) + K=48 at f0+2*258 (dy=2), accumulating in PSUM.
  - Layer2: one K=128 matmul per pair, N=1024, M=32, at column tile_position
    (0, 32q) -> the four pairs of a block land on PSUM partition strips
    0/32/64/96 of one [128,1024] tile and can run on independent column
    tiles of the PE array.
  - Post-ops on 128 partitions: u = pdx + xres (xres = x+b2, host-prepped,
    strip layout), o = min(max(u,0),1); bf16 output, host converts to f32.
  - Layer2+postops for a pair are emitted one pair later than its layer1 so
    the PE never waits on the relu; relu alternates ACT/DVE engines.
"""

import numpy as np
from contextlib import ExitStack

import concourse.bass as bass
import concourse.tile as tile
from concourse import bacc, mybir
from concourse.bass_utils import run_bass_kernel_spmd

B, S, C, HID = 8, 256, 16, 128
N_CORES = 8
P = S + 2                    # padded width = 258
FLAT = P * P                 # 66564
CH = 1024                    # pixels per chunk (pair granularity)
BF = 4                       # chunks per block
NCHUNK = 65                  # 65*1024 = 66560 covers all valid positions
XLEN = 72 * CH + 2 * P + 16  # padded flat length

_CACHE = {}


def _build_v2(bf=BF, xx_bufs=3, h_bufs=4, o_bufs=3, r_bufs=3,
              ph_bufs=2, pdx_bufs=2, n1024=False):
    f32 = mybir.dt.float32
    bf16 = mybir.dt.bfloat16
    Relu = mybir.ActivationFunctionType.Relu
    add = mybir.AluOpType.add
    op_max = mybir.AluOpType.max
    op_min = mybir.AluOpType.min

    nc = bacc.Bacc("TRN2", target_bir_lowering=False, debug=False,
                   num_devices=N_CORES)

    xfb = nc.dram_tensor("xfb", [C, XLEN], bf16, kind="ExternalInput").ap()
    xrb = nc.dram_tensor("xrb", [C, XLEN], bf16, kind="ExternalInput").ap()
    wc96 = nc.dram_tensor("wc96", [96, HID], bf16, kind="ExternalInput").ap()
    wc48 = nc.dram_tensor("wc48", [48, HID], bf16, kind="ExternalInput").ap()
    w2 = nc.dram_tensor("w2", [HID, 32], bf16, kind="ExternalInput").ap()
    b1 = nc.dram_tensor("b1", [HID, 1], f32, kind="ExternalInput").ap()
    out = nc.dram_tensor("out", [C, XLEN], bf16, kind="ExternalOutput").ap()

    nblocks = (NCHUNK + bf - 1) // bf
    span = bf * CH + 2 * P + 8

    with tile.TileContext(nc) as tc, ExitStack() as ctx:
        wpool = ctx.enter_context(tc.tile_pool(name="wts", bufs=1))
        wc96_sb = wpool.tile([96, HID], bf16)
        nc.sync.dma_start(wc96_sb[:], wc96)
        wc48_sb = wpool.tile([48, HID], bf16)
        nc.sync.dma_start(wc48_sb[:], wc48)
        w2_sb = wpool.tile([HID, 32], bf16)
        nc.sync.dma_start(w2_sb[:], w2)
        b1_sb = wpool.tile([HID, 1], f32)
        nc.sync.dma_start(b1_sb[:], b1)

        xpool = ctx.enter_context(tc.tile_pool(name="xx", bufs=xx_bufs))
        rpool = ctx.enter_context(tc.tile_pool(name="xres", bufs=r_bufs))
        hpool = ctx.enter_context(tc.tile_pool(name="h", bufs=h_bufs))
        upool = ctx.enter_context(tc.tile_pool(name="u", bufs=2))
        opool = ctx.enter_context(tc.tile_pool(name="o", bufs=o_bufs))
        ph_pool = ctx.enter_context(
            tc.tile_pool(name="ph", bufs=ph_bufs, space="PSUM"))
        pdx_pool = ctx.enter_context(
            tc.tile_pool(name="pdx", bufs=pdx_bufs, space="PSUM"))

        def load_block(b):
            q0 = b * bf * CH
            xx = xpool.tile([96, span], bf16, tag="xx")
            for dy in range(2):
                base = xfb[:, q0 + dy * P:q0 + dy * P + span]
                src = bass.AP(tensor=base.tensor, offset=base.offset,
                              ap=[[1, 3]] + [list(p) for p in base.ap])
                nc.sync.dma_start(xx[48 * dy:48 * dy + 48, :], src)
            xres = rpool.tile([128, CH], bf16, tag="xres")
            for q in range(bf):
                off = q0 + P + 1 + q * CH
                nc.sync.dma_start(xres[32 * q:32 * q + C, :],
                                  xrb[:, off:off + CH])
            return xx, xres

        # pipeline state
        blk = [None] * nblocks   # per block: dict(pdx, xres, o, done pairs)
        xx_cur, xres_cur = load_block(0)

        def do_l1(xx, q):
            f0 = q * CH
            ph = ph_pool.tile([HID, CH], f32, tag="ph")
            if n1024:
                nc.tensor.matmul(ph[:], lhsT=wc96_sb[:],
                                 rhs=xx[0:96, f0:f0 + CH],
                                 start=True, stop=False)
                nc.tensor.matmul(ph[:], lhsT=wc48_sb[:],
                                 rhs=xx[0:48, f0 + 2 * P:f0 + 2 * P + CH],
                                 start=False, stop=True)
            else:
                hc = CH // 2
                for s2 in range(2):
                    g0 = f0 + s2 * hc
                    nc.tensor.matmul(ph[:, s2 * hc:(s2 + 1) * hc],
                                     lhsT=wc96_sb[:],
                                     rhs=xx[0:96, g0:g0 + hc],
                                     start=True, stop=False)
                    nc.tensor.matmul(ph[:, s2 * hc:(s2 + 1) * hc],
                                     lhsT=wc48_sb[:],
                                     rhs=xx[0:48, g0 + 2 * P:g0 + 2 * P + hc],
                                     start=False, stop=True)
            return ph

        def do_relu(ph, q):
            h = hpool.tile([HID, CH], bf16, tag="h")
            if q % 2 == 0:
                nc.scalar.activation(h[:], ph[:], Relu, bias=b1_sb[:])
            else:
                nc.vector.tensor_scalar(h[:], ph[:], b1_sb[:], 0.0,
                                        op0=add, op1=op_max)
            return h

        def do_l2(st, q, h):
            if n1024:
                nc.tensor.matmul(st["pdx"][32 * q:32 * q + 32, :],
                                 lhsT=w2_sb[:], rhs=h[:],
                                 start=True, stop=True,
                                 tile_position=(0, 32 * q))
            else:
                hc = CH // 2
                for s2 in range(2):
                    nc.tensor.matmul(
                        st["pdx"][32 * q:32 * q + 32, s2 * hc:(s2 + 1) * hc],
                        lhsT=w2_sb[:], rhs=h[:, s2 * hc:(s2 + 1) * hc],
                        start=True, stop=True,
                        tile_position=(0, 32 * q))
            st["n"] += 1
            if st["n"] == bf:
                u = upool.tile([128, CH], f32, tag="u")
                nc.vector.tensor_tensor(u[:], st["pdx"][:], st["xres"][:],
                                        op=add)
                o = opool.tile([128, CH], bf16, tag="o")
                nc.vector.tensor_scalar(o[:], u[:], 0.0, 1.0,
                                        op0=op_max, op1=op_min)
                p0 = st["b"] * bf * CH
                for q2 in range(bf):
                    off = p0 + P + 1 + q2 * CH
                    nc.sync.dma_start(out[:, off:off + CH],
                                      o[32 * q2:32 * q2 + C, :])

        pend = None  # (st, q, h) waiting one pair before layer2
        for b in range(nblocks):
            pdx_t = pdx_pool.tile([128, CH], f32, tag="pdx")
            st = {"pdx": pdx_t, "xres": xres_cur, "n": 0, "b": b}
            nxt = load_block(b + 1) if b + 1 < nblocks else (None, None)
            for q in range(bf):
                ph = do_l1(xx_cur, q)
                h = do_relu(ph, q)
                if pend is not None:
                    do_l2(*pend)
                pend = (st, q, h)
            xx_cur, xres_cur = nxt
        do_l2(*pend)

    nc.compile()
    return nc


def _prep_weights(pk, W1):
    # pk [3(dy),3(dx),3(k)]; W1 [48,128] rows indexed 3*ci+k
    W1r = W1.reshape(C, 3, HID)                      # [ci, k, hid]
    Wfull = np.einsum("ydk,ckh->ydch", pk, W1r)      # [dy, dx, ci, hid]
    wc96 = np.concatenate([Wfull[0].reshape(3 * C, HID),
                           Wfull[1].reshape(3 * C, HID)], axis=0)
    wc48 = Wfull[2].reshape(3 * C, HID)
    return wc96, wc48


def kernel(x, perception_kernel, W1, b1, W2, b2):
    import ml_dtypes
    bf16 = ml_dtypes.bfloat16
    x = np.asarray(x, dtype=np.float32)
    pk = np.asarray(perception_kernel, dtype=np.float32)
    W1 = np.asarray(W1, dtype=np.float32)
    b1 = np.asarray(b1, dtype=np.float32)
    W2 = np.asarray(W2, dtype=np.float32)
    b2 = np.asarray(b2, dtype=np.float32)

    if "nc" not in _CACHE:
        _CACHE["nc"] = _build_v2()
    nc = _CACHE["nc"]

    wc96, wc48 = _prep_weights(pk, W1)
    w2_np = np.zeros((HID, 32), np.float32)
    w2_np[:, :C] = W2
    b1_np = np.ascontiguousarray(b1.reshape(HID, 1))

    in_maps = []
    for c in range(N_CORES):
        xt = np.ascontiguousarray(x[c].transpose(2, 0, 1))      # [C, S, S]
        xt = np.pad(xt, ((0, 0), (1, 1), (1, 1)), mode="wrap")  # [C, 258, 258]
        xflat = np.zeros((C, XLEN), np.float32)
        xflat[:, :FLAT] = xt.reshape(C, FLAT)
        # residual carries +b2 folded in (b2 broadcast over channels dim 0)
        xres_f = xflat + b2.reshape(C, 1)
        in_maps.append({
            "xfb": xflat.astype(bf16),
            "xrb": xres_f.astype(bf16),
            "wc96": wc96.astype(bf16), "wc48": wc48.astype(bf16),
            "w2": w2_np.astype(bf16), "b1": b1_np,
        })

    import os as _os
    _trace = bool(int(_os.environ.get("KTRACE", "0")))
    if _trace:
        import tempfile as _tempfile
        from trn_agent_boot.trn_boot import _ntff_profile_via_ctypes
        _hook = _ntff_profile_via_ctypes('/opt/axon/libaxon_pjrt.so')
        _neff_dir = _tempfile.mkdtemp(prefix="ktrace_")
        with _hook(_neff_dir, [0]):
            res = run_bass_kernel_spmd(nc, in_maps, list(range(N_CORES)))
        _CACHE["neff_dir"] = _neff_dir
        _CACHE["nc_obj"] = nc
    else:
        res = run_bass_kernel_spmd(nc, in_maps, list(range(N_CORES)))
    _CACHE["exec_time_ns"] = getattr(res, "exec_time_ns", None)
    _CACHE["trace"] = getattr(res, "instructions_and_trace", None)
    outs = []
    for c in range(N_CORES):
        of = res.results[c]["out"][:, :FLAT].astype(np.float32)
        of = of.reshape(C, P, P)
        outs.append(of[:, 1:S + 1, 1:S + 1].transpose(1, 2, 0))
    return np.ascontiguousarray(np.stack(outs, axis=0), dtype=np.float32)


# revision 14
# speedup vs baseline: 1.3264x; 1.3264x over previous
"""Trainium2 Bass kernel for the CellularAutomata step (dense_cnn) — v2.

Math (per pixel): s = depthwise3x3(wrap_pad(x), [identity, sobel_x, sobel_y]);
h = relu(s @ W1 + b1); out = clip(x + h @ W2 + b2, 0, 1).

Strategy (pure data parallel, batch -> 8 cores, weights replicated):
  - Host: per-core image to channel-major flat layout [16, 258*258] with wrap
    padding (bf16); the device computes the padded flat grid and the host
    slices out the valid 256x256 region.
  - The 3x3 conv + W1 are folded host-side.  xx holds 6 shifted copies of x
    on 96 partitions: partition dy*48+dx*16+c = x[c, q0+dy*258+dx+f], loaded
    in one DMA.  Layer1 for a 1024-px pair is TWO bf16 matmuls: K=96 at
    column f0 (taps dy=0,1) + K=48 at f0+2*258 (dy=2), accumulating in PSUM.
  - Layer2: one K=128 matmul per pair, N=1024, M=32, at column tile_position
    (0, 32q) -> the four pairs of a block land on PSUM partition strips
    0/32/64/96 of one [128,1024] tile and can run on independent column
    tiles of the PE array.
  - Post-ops on 128 partitions: u = pdx + xres (xres = x+b2, host-prepped,
    strip layout), o = min(max(u,0),1); bf16 output, host converts to f32.
  - Layer2+postops for a pair are emitted one pair later than its layer1 so
    the PE never waits on the relu; relu alternates ACT/DVE engines.
"""

import numpy as np
from contextlib import ExitStack

import concourse.bass as bass
import concourse.tile as tile
from concourse import bacc, mybir
from concourse.bass_utils import run_bass_kernel_spmd

B, S, C, HID = 8, 256, 16, 128
N_CORES = 8
P = S + 2                    # padded width = 258
FLAT = P * P                 # 66564
CH = 1024                    # pixels per chunk (pair granularity)
BF = 4                       # chunks per block
NCHUNK = 65                  # 65*1024 = 66560 covers all valid positions
XLEN = 72 * CH + 2 * P + 16  # padded flat length

_CACHE = {}


def _build_v2(bf=BF, xx_bufs=4, h_bufs=4, o_bufs=3, r_bufs=4,
              ph_bufs=2, pdx_bufs=2, n1024=False):
    f32 = mybir.dt.float32
    bf16 = mybir.dt.bfloat16
    Relu = mybir.ActivationFunctionType.Relu
    add = mybir.AluOpType.add
    op_max = mybir.AluOpType.max
    op_min = mybir.AluOpType.min

    nc = bacc.Bacc("TRN2", target_bir_lowering=False, debug=False,
                   num_devices=N_CORES)

    xfb = nc.dram_tensor("xfb", [C, XLEN], bf16, kind="ExternalInput").ap()
    xrb = nc.dram_tensor("xrb", [C, XLEN], bf16, kind="ExternalInput").ap()
    wc96 = nc.dram_tensor("wc96", [96, HID], bf16, kind="ExternalInput").ap()
    wc48 = nc.dram_tensor("wc48", [48, HID], bf16, kind="ExternalInput").ap()
    w2 = nc.dram_tensor("w2", [HID, 32], bf16, kind="ExternalInput").ap()
    b1 = nc.dram_tensor("b1", [HID, 1], f32, kind="ExternalInput").ap()
    out = nc.dram_tensor("out", [C, XLEN], bf16, kind="ExternalOutput").ap()

    nblocks = (NCHUNK + bf - 1) // bf
    span = bf * CH + 2 * P + 8

    with tile.TileContext(nc) as tc, ExitStack() as ctx:
        wpool = ctx.enter_context(tc.tile_pool(name="wts", bufs=1))
        wc96_sb = wpool.tile([96, HID], bf16)
        nc.sync.dma_start(wc96_sb[:], wc96)
        wc48_sb = wpool.tile([48, HID], bf16)
        nc.sync.dma_start(wc48_sb[:], wc48)
        w2_sb = wpool.tile([HID, 32], bf16)
        nc.sync.dma_start(w2_sb[:], w2)
        b1_sb = wpool.tile([HID, 1], f32)
        nc.sync.dma_start(b1_sb[:], b1)

        xpool = ctx.enter_context(tc.tile_pool(name="xx", bufs=xx_bufs))
        rpool = ctx.enter_context(tc.tile_pool(name="xres", bufs=r_bufs))
        hpool = ctx.enter_context(tc.tile_pool(name="h", bufs=h_bufs))
        upool = ctx.enter_context(tc.tile_pool(name="u", bufs=2))
        opool = ctx.enter_context(tc.tile_pool(name="o", bufs=o_bufs))
        ph_pool = ctx.enter_context(
            tc.tile_pool(name="ph", bufs=ph_bufs, space="PSUM"))
        pdx_pool = ctx.enter_context(
            tc.tile_pool(name="pdx", bufs=pdx_bufs, space="PSUM"))

        def load_block(b):
            q0 = b * bf * CH
            xx = xpool.tile([96, span], bf16, tag="xx")
            for dy in range(2):
                base = xfb[:, q0 + dy * P:q0 + dy * P + span]
                src = bass.AP(tensor=base.tensor, offset=base.offset,
                              ap=[[1, 3]] + [list(p) for p in base.ap])
                nc.sync.dma_start(xx[48 * dy:48 * dy + 48, :], src)
            xres = rpool.tile([128, CH], bf16, tag="xres")
            for q in range(bf):
                off = q0 + P + 1 + q * CH
                nc.scalar.dma_start(xres[32 * q:32 * q + C, :],
                                    xrb[:, off:off + CH])
            return xx, xres

        # pipeline state: 2-block DMA lookahead
        pre = [load_block(0), load_block(1) if nblocks > 1 else (None, None)]

        def do_l1(xx, q):
            f0 = q * CH
            ph = ph_pool.tile([HID, CH], f32, tag="ph")
            if n1024:
                nc.tensor.matmul(ph[:], lhsT=wc96_sb[:],
                                 rhs=xx[0:96, f0:f0 + CH],
                                 start=True, stop=False)
                nc.tensor.matmul(ph[:], lhsT=wc48_sb[:],
                                 rhs=xx[0:48, f0 + 2 * P:f0 + 2 * P + CH],
                                 start=False, stop=True)
            else:
                # same lhsT back-to-back so LDWEIGHTS can be reused/hidden
                hc = CH // 2
                for s2 in range(2):
                    g0 = f0 + s2 * hc
                    nc.tensor.matmul(ph[:, s2 * hc:(s2 + 1) * hc],
                                     lhsT=wc96_sb[:],
                                     rhs=xx[0:96, g0:g0 + hc],
                                     start=True, stop=False)
                for s2 in range(2):
                    g0 = f0 + s2 * hc
                    nc.tensor.matmul(ph[:, s2 * hc:(s2 + 1) * hc],
                                     lhsT=wc48_sb[:],
                                     rhs=xx[0:48, g0 + 2 * P:g0 + 2 * P + hc],
                                     start=False, stop=True)
            return ph

        def do_relu(ph, q):
            h = hpool.tile([HID, CH], bf16, tag="h")
            if q % 2 == 0:
                nc.scalar.activation(h[:], ph[:], Relu, bias=b1_sb[:])
            else:
                nc.vector.tensor_scalar(h[:], ph[:], b1_sb[:], 0.0,
                                        op0=add, op1=op_max)
            return h

        def do_l2(st, q, h):
            if n1024:
                nc.tensor.matmul(st["pdx"][32 * q:32 * q + 32, :],
                                 lhsT=w2_sb[:], rhs=h[:],
                                 start=True, stop=True,
                                 tile_position=(0, 32 * q))
            else:
                hc = CH // 2
                for s2 in range(2):
                    nc.tensor.matmul(
                        st["pdx"][32 * q:32 * q + 32, s2 * hc:(s2 + 1) * hc],
                        lhsT=w2_sb[:], rhs=h[:, s2 * hc:(s2 + 1) * hc],
                        start=True, stop=True,
                        tile_position=(0, 32 * q))
            st["n"] += 1
            if st["n"] == bf:
                u = upool.tile([128, CH], f32, tag="u")
                nc.vector.tensor_tensor(u[:], st["pdx"][:], st["xres"][:],
                                        op=add)
                o = opool.tile([128, CH], bf16, tag="o")
                nc.vector.tensor_scalar(o[:], u[:], 0.0, 1.0,
                                        op0=op_max, op1=op_min)
                p0 = st["b"] * bf * CH
                for q2 in range(bf):
                    off = p0 + P + 1 + q2 * CH
                    nc.gpsimd.dma_start(out[:, off:off + CH],
                                        o[32 * q2:32 * q2 + C, :])

        pend = None  # (st, q, h) waiting one pair before layer2
        for b in range(nblocks):
            xx_cur, xres_cur = pre[0]
            pdx_t = pdx_pool.tile([128, CH], f32, tag="pdx")
            st = {"pdx": pdx_t, "xres": xres_cur, "n": 0, "b": b}
            nxt = load_block(b + 2) if b + 2 < nblocks else (None, None)
            pre = [pre[1], nxt]
            for q in range(bf):
                ph = do_l1(xx_cur, q)
                h = do_relu(ph, q)
                if pend is not None:
                    do_l2(*pend)
                pend = (st, q, h)
        do_l2(*pend)

    nc.compile()
    return nc


def _prep_weights(pk, W1):
    # pk [3(dy),3(dx),3(k)]; W1 [48,128] rows indexed 3*ci+k
    W1r = W1.reshape(C, 3, HID)                      # [ci, k, hid]
    Wfull = np.einsum("ydk,ckh->ydch", pk, W1r)      # [dy, dx, ci, hid]
    wc96 = np.concatenate([Wfull[0].reshape(3 * C, HID),
                           Wfull[1].reshape(3 * C, HID)], axis=0)
    wc48 = Wfull[2].reshape(3 * C, HID)
    return wc96, wc48


def kernel(x, perception_kernel, W1, b1, W2, b2):
    import ml_dtypes
    bf16 = ml_dtypes.bfloat16
    x = np.asarray(x, dtype=np.float32)
    pk = np.asarray(perception_kernel, dtype=np.float32)
    W1 = np.asarray(W1, dtype=np.float32)
    b1 = np.asarray(b1, dtype=np.float32)
    W2 = np.asarray(W2, dtype=np.float32)
    b2 = np.asarray(b2, dtype=np.float32)

    if "nc" not in _CACHE:
        _CACHE["nc"] = _build_v2()
    nc = _CACHE["nc"]

    wc96, wc48 = _prep_weights(pk, W1)
    w2_np = np.zeros((HID, 32), np.float32)
    w2_np[:, :C] = W2
    b1_np = np.ascontiguousarray(b1.reshape(HID, 1))

    in_maps = []
    for c in range(N_CORES):
        xt = np.ascontiguousarray(x[c].transpose(2, 0, 1))      # [C, S, S]
        xt = np.pad(xt, ((0, 0), (1, 1), (1, 1)), mode="wrap")  # [C, 258, 258]
        xflat = np.zeros((C, XLEN), np.float32)
        xflat[:, :FLAT] = xt.reshape(C, FLAT)
        # residual carries +b2 folded in (b2 broadcast over channels dim 0)
        xres_f = xflat + b2.reshape(C, 1)
        in_maps.append({
            "xfb": xflat.astype(bf16),
            "xrb": xres_f.astype(bf16),
            "wc96": wc96.astype(bf16), "wc48": wc48.astype(bf16),
            "w2": w2_np.astype(bf16), "b1": b1_np,
        })

    import os as _os
    _trace = bool(int(_os.environ.get("KTRACE", "0")))
    if _trace:
        import tempfile as _tempfile
        from trn_agent_boot.trn_boot import _ntff_profile_via_ctypes
        _hook = _ntff_profile_via_ctypes('/opt/axon/libaxon_pjrt.so')
        _neff_dir = _tempfile.mkdtemp(prefix="ktrace_")
        with _hook(_neff_dir, [0]):
            res = run_bass_kernel_spmd(nc, in_maps, list(range(N_CORES)))
        _CACHE["neff_dir"] = _neff_dir
        _CACHE["nc_obj"] = nc
    else:
        res = run_bass_kernel_spmd(nc, in_maps, list(range(N_CORES)))
    _CACHE["exec_time_ns"] = getattr(res, "exec_time_ns", None)
    _CACHE["trace"] = getattr(res, "instructions_and_trace", None)
    outs = []
    for c in range(N_CORES):
        of = res.results[c]["out"][:, :FLAT].astype(np.float32)
        of = of.reshape(C, P, P)
        outs.append(of[:, 1:S + 1, 1:S + 1].transpose(1, 2, 0))
    return np.ascontiguousarray(np.stack(outs, axis=0), dtype=np.float32)
